# revision 9
# baseline (speedup 1.0000x reference)
"""Self-contained Trainium2 Bass kernel for the "Attentive" GNN message-passing
problem:

    x: [8192, 256] f32, attn_vectors: [4, 256] f32
    e_h = l2_normalize(attn_vectors[h] * x, axis=-1)        # [H, N, D]
    Y   = concat_h(e_h)                                     # [N, H*D]
    out = (Y @ Y.T) / H                                     # [N, N]

Strategy (8 NeuronCores, SPMD, no collectives):
  - Output rows are sharded 8 x 1024. Every core receives the FULL x plus its
    own x_local row-shard as separate inputs, so the program is core-agnostic.
  - Everything on-chip runs in "features on partitions" layout (x^T), so the
    row-wise L2 norm becomes a cross-partition reduction done on the PE:
        s_h[n] = sum_d attn[h,d]^2 * x[d,n]^2   (tiny matmul, all 4 heads)
    rnorm = 0.5 / sqrt(max(s, eps))  (the 1/H=1/4 output scale is folded in
    as 0.5 per side; ACT Sqrt + DVE reciprocal since ACT Rsqrt is inaccurate).
  - Y^T never touches DRAM: each 512-column panel of Y^T is built in SBUF
    (PE outer-product broadcasts a_h[d]*rnorm_h[n], one DVE multiply per
    128-row k-chunk) and consumed by 8x8 accumulating 128x512 matmuls.
  - lhsT (Y^T restricted to the core's 1024 local rows) is built once and
    stays resident in SBUF.
"""

from contextlib import ExitStack

import numpy as np

N, D, H = 8192, 256, 4
NCORES = 8
NLOC = N // NCORES  # 1024 output rows per core
P = 128
PANEL = 512
NPANELS = N // PANEL  # 16
RBLK = NLOC // P  # 8 row blocks of the local output
KCH = (H * D) // P  # 8 contraction chunks of 128
CHD = D // P  # 2 chunks per head
EPS = 1e-12

_COMPILED = {}


def _build_bass():
    import concourse.bass as bass  # noqa: F401
    import concourse.tile as tile
    from concourse import bacc, mybir
    from concourse.masks import make_identity

    f32 = mybir.dt.float32

    nc = bacc.Bacc(
        "TRN2",
        target_bir_lowering=False,
        debug=False,
        enable_asserts=False,
        num_devices=NCORES,
    )
    x_t = nc.dram_tensor("x", [N, D], f32, kind="ExternalInput")
    xl_t = nc.dram_tensor("x_local", [NLOC, D], f32, kind="ExternalInput")
    # Host-precomputed functions of attn_vectors (tiny):
    #   a_mask[h', kc*128+d] = (h'==kc//2) ? 0.5*attn[h', (kc%2)*128+d] : 0
    #   w_sq[d, c*4+h]       = attn[h, c*128+d]^2
    am_t = nc.dram_tensor("a_mask", [H, KCH * P], f32, kind="ExternalInput")
    ws_t = nc.dram_tensor("w_sq", [P, CHD * H], f32, kind="ExternalInput")
    out_t = nc.dram_tensor("out", [NLOC, N], f32, kind="ExternalOutput")

    x, xl, out = x_t.ap(), xl_t.ap(), out_t.ap()

    with tile.TileContext(nc) as tc, ExitStack() as ctx:
        consts = ctx.enter_context(tc.tile_pool(name="consts", bufs=1))
        loads = ctx.enter_context(tc.tile_pool(name="loads", bufs=4))
        xtp = ctx.enter_context(tc.tile_pool(name="xtp", bufs=3))
        xlocp = ctx.enter_context(tc.tile_pool(name="xlocp", bufs=1))
        sq = ctx.enter_context(tc.tile_pool(name="sq", bufs=2))
        small = ctx.enter_context(tc.tile_pool(name="small", bufs=2))
        rhsp = ctx.enter_context(tc.tile_pool(name="rhsp", bufs=2))
        outp = ctx.enter_context(tc.tile_pool(name="outp", bufs=3))
        ps_tp = ctx.enter_context(
            tc.tile_pool(name="ps_tp", bufs=2, space="PSUM")
        )
        ps_norm = ctx.enter_context(
            tc.tile_pool(name="ps_norm", bufs=2, space="PSUM")
        )
        ps_outer = ctx.enter_context(
            tc.tile_pool(name="ps_outer", bufs=2, space="PSUM")
        )
        ps_out = ctx.enter_context(
            tc.tile_pool(name="ps_out", bufs=2, space="PSUM")
        )

        # ---- constants -----------------------------------------------------
        ident = consts.tile([P, P], f32)
        make_identity(nc, ident[:])

        a_mask = consts.tile([H, KCH * P], f32)
        nc.sync.dma_start(a_mask[:], am_t.ap()[:])
        w_sq = consts.tile([P, CHD * H], f32)
        nc.sync.dma_start(w_sq[:], ws_t.ap()[:])

        # ---- helpers -------------------------------------------------------
        copy_flip = [0]

        def load_and_transpose(src_ap, row0, dst_tile, i):
            """Rows [row0, row0+128) of src into dst_tile[:, c*512 + i*128]."""
            xt = loads.tile([P, D], f32, tag="xload")
            nc.sync.dma_start(xt[:], src_ap[row0 : row0 + P, :])
            for c in range(CHD):
                pt = ps_tp.tile([P, P], f32, tag="tp")
                nc.tensor.transpose(pt[:], xt[:, c * P : (c + 1) * P], ident[:])
                dst = dst_tile[:, c * PANEL + i * P : c * PANEL + (i + 1) * P]
                if copy_flip[0] % 2 == 0:
                    nc.vector.tensor_copy(dst, pt[:])
                else:
                    nc.scalar.copy(dst, pt[:])
                copy_flip[0] += 1

        def build_rnorm(xT_tile):
            """[4, 512] tile of 0.5/sqrt(max(s,eps)) for this panel."""
            pn = ps_norm.tile([H, PANEL], f32)
            for c in range(CHD):
                xsq = sq.tile([P, PANEL], f32, tag="xsq")
                src = xT_tile[:, c * PANEL : (c + 1) * PANEL]
                nc.vector.tensor_mul(xsq[:], src, src)
                nc.tensor.matmul(
                    pn[:],
                    w_sq[:, c * H : (c + 1) * H],
                    xsq[:],
                    start=(c == 0),
                    stop=(c == CHD - 1),
                )
            clamped = small.tile([H, PANEL], f32, tag="clamped")
            nc.vector.tensor_scalar_max(clamped[:], pn[:], EPS)
            root = small.tile([H, PANEL], f32, tag="root")
            nc.scalar.sqrt(root[:], clamped[:])
            rnorm = small.tile([H, PANEL], f32, tag="rnorm")
            nc.vector.reciprocal(rnorm[:], root[:])
            return rnorm

        # ---- local rows: build resident lhsT -------------------------------
        lhsT = consts.tile([P, KCH * NLOC], f32)
        xlocT = []
        for lp in range(2):
            t = xlocp.tile([P, CHD * PANEL], f32, tag=f"xlocT{lp}")
            xlocT.append(t)
            for i in range(PANEL // P):
                load_and_transpose(xl, lp * PANEL + i * P, t, i)

        for lp in range(2):
            rn = build_rnorm(xlocT[lp])
            for kc in range(KCH):
                h, c = divmod(kc, CHD)
                po = ps_outer.tile([P, PANEL], f32)
                nc.tensor.matmul(
                    po[:],
                    a_mask[:, kc * P : (kc + 1) * P],
                    rn[:],
                    start=True,
                    stop=True,
                )
                nc.vector.tensor_mul(
                    lhsT[:, kc * NLOC + lp * PANEL : kc * NLOC + (lp + 1) * PANEL],
                    xlocT[lp][:, c * PANEL : (c + 1) * PANEL],
                    po[:],
                )

        # ---- main loop over 16 column panels -------------------------------
        for p in range(NPANELS):
            xT = xtp.tile([P, CHD * PANEL], f32, tag="xT")
            for i in range(PANEL // P):
                load_and_transpose(x, p * PANEL + i * P, xT, i)

            rn = build_rnorm(xT)

            rhs = rhsp.tile([P, KCH * PANEL], f32)
            for kc in range(KCH):
                h, c = divmod(kc, CHD)
                po = ps_outer.tile([P, PANEL], f32)
                nc.tensor.matmul(
                    po[:],
                    a_mask[:, kc * P : (kc + 1) * P],
                    rn[:],
                    start=True,
                    stop=True,
                )
                nc.vector.tensor_mul(
                    rhs[:, kc * PANEL : (kc + 1) * PANEL],
                    xT[:, c * PANEL : (c + 1) * PANEL],
                    po[:],
                )

            for r in range(RBLK):
                acc = ps_out.tile([P, PANEL], f32)
                for kc in range(KCH):
                    nc.tensor.matmul(
                        acc[:],
                        lhsT[:, kc * NLOC + r * P : kc * NLOC + (r + 1) * P],
                        rhs[:, kc * PANEL : (kc + 1) * PANEL],
                        start=(kc == 0),
                        stop=(kc == KCH - 1),
                    )
                ot = outp.tile([P, PANEL], f32)
                nc.scalar.copy(ot[:], acc[:])
                nc.sync.dma_start(
                    out[r * P : (r + 1) * P, p * PANEL : (p + 1) * PANEL], ot[:]
                )

    nc.compile()
    return nc


def _get_compiled():
    if "nc" not in _COMPILED:
        _COMPILED["nc"] = _build_bass()
    return _COMPILED["nc"]


def host_side_inputs(x, attn):
    """Per-core input maps (a_mask / w_sq are tiny host-precomputed
    functions of attn_vectors; see _build_bass)."""
    a_mask = np.zeros((H, KCH * P), dtype=np.float32)
    for kc in range(KCH):
        h, c = divmod(kc, CHD)
        a_mask[h, kc * P : (kc + 1) * P] = 0.5 * attn[h, c * P : (c + 1) * P]
    w_sq = np.zeros((P, CHD * H), dtype=np.float32)
    for c in range(CHD):
        w_sq[:, c * H : (c + 1) * H] = (attn[:, c * P : (c + 1) * P] ** 2).T
    return [
        {
            "x": x,
            "x_local": np.ascontiguousarray(x[c * NLOC : (c + 1) * NLOC]),
            "a_mask": a_mask,
            "w_sq": w_sq,
        }
        for c in range(NCORES)
    ]


def kernel(**inputs) -> np.ndarray:
    from concourse import bass_utils

    x = np.ascontiguousarray(np.asarray(inputs["x"], dtype=np.float32))
    attn = np.ascontiguousarray(
        np.asarray(inputs["attn_vectors"], dtype=np.float32)
    )
    nc = _get_compiled()
    res = bass_utils.run_bass_kernel_spmd(
        nc, host_side_inputs(x, attn), core_ids=list(range(NCORES))
    )
    return np.concatenate([r["out"] for r in res.results], axis=0)


# revision 14
# speedup vs baseline: 2.8473x; 2.8473x over previous
"""Self-contained Trainium2 Bass kernel for the "Attentive" GNN message-passing
problem:

    x: [8192, 256] f32, attn_vectors: [4, 256] f32
    e_h = l2_normalize(attn_vectors[h] * x, axis=-1)        # [H, N, D]
    Y   = concat_h(e_h)                                     # [N, H*D]
    out = (Y @ Y.T) / H                                     # [N, N]

Strategy (8 NeuronCores, SPMD, no collectives):
  - Output rows are sharded 8 x 1024; every core receives the FULL x plus its
    own x_local row-shard as separate inputs, so the program is core-agnostic.
  - Key algebra: out[i,j] = sum_k (x*a^2*r/H)[i,k] * (x*r)[j,k] with
    r_h[n] = 1/sqrt(max(sum_d (a_h[d]*x[n,d])^2, eps)); a^2 and the 1/H are
    folded into the (small, resident) lhsT side only, so the streamed rhs
    panels need just one elementwise multiply each.
  - Everything runs in "features on partitions" layout (x^T), obtained by
    staging a bf16 copy of x in DRAM and reading it back through the DMA
    xbar transpose (f32 DMA transpose is unsupported; bf16 rounding of the
    matmul operands is the precision budget anyway).
  - Row norms are cross-partition sums done as tiny PE matmuls against
    host-precomputed a^2 weights; rnorm stays f32 and is broadcast across
    partitions by the (otherwise idle) GpSimd engine.
  - Matmul inputs are bf16 (PE runs f32 at half rate); PSUM accumulates f32
    and results are DMA'd straight from PSUM to DRAM.
"""

from contextlib import ExitStack

import numpy as np

N, D, H = 8192, 256, 4
NCORES = 8
NLOC = N // NCORES  # 1024 output rows per core
P = 128
PANEL = 512
NPANELS = N // PANEL  # 16
RBLK = NLOC // P  # 8 row blocks of the local output
KCH = (H * D) // P  # 8 contraction chunks of 128
CHD = D // P  # 2 chunks per head
EPS = 1e-12

_COMPILED = {}


def _build_bass():
    import concourse.bass as bass  # noqa: F401
    import concourse.tile as tile
    from concourse import bacc, mybir

    f32 = mybir.dt.float32
    bf16 = mybir.dt.bfloat16

    nc = bacc.Bacc(
        "TRN2",
        target_bir_lowering=False,
        debug=False,
        enable_asserts=False,
        num_devices=NCORES,
    )
    x_t = nc.dram_tensor("x", [N, D], f32, kind="ExternalInput")
    xl_t = nc.dram_tensor("x_local", [NLOC, D], f32, kind="ExternalInput")
    # Host-precomputed functions of attn_vectors (tiny):
    #   w_sq[d, c*4+h]  = attn[h, c*128+d]^2          (bf16, norm matmul lhsT)
    #   asq[d, kc]      = 0.25*attn[h, c*128+d]^2     (f32, kc = h*2+c)
    ws_t = nc.dram_tensor("w_sq", [P, CHD * H], bf16, kind="ExternalInput")
    aq_t = nc.dram_tensor("asq", [P, KCH], f32, kind="ExternalInput")
    out_t = nc.dram_tensor("out", [NLOC, N], f32, kind="ExternalOutput")

    x, xl, out = x_t.ap(), xl_t.ap(), out_t.ap()

    with tile.TileContext(nc) as tc, ExitStack() as ctx:
        consts = ctx.enter_context(tc.tile_pool(name="consts", bufs=1))
        loads = ctx.enter_context(tc.tile_pool(name="loads", bufs=4))
        xtp = ctx.enter_context(tc.tile_pool(name="xtp", bufs=3))
        xlocp = ctx.enter_context(tc.tile_pool(name="xlocp", bufs=1))
        sq = ctx.enter_context(tc.tile_pool(name="sq", bufs=2))
        small = ctx.enter_context(tc.tile_pool(name="small", bufs=2))
        bcp = ctx.enter_context(tc.tile_pool(name="bcp", bufs=3))
        rhsp = ctx.enter_context(tc.tile_pool(name="rhsp", bufs=3))
        outp = ctx.enter_context(tc.tile_pool(name="outp", bufs=4))
        dram = ctx.enter_context(tc.tile_pool(name="dram", bufs=1, space="DRAM"))
        ps_norm = ctx.enter_context(
            tc.tile_pool(name="ps_norm", bufs=2, space="PSUM")
        )
        ps_out = ctx.enter_context(
            tc.tile_pool(name="ps_out", bufs=4, space="PSUM")
        )

        w_sq = consts.tile([P, CHD * H], bf16)
        nc.sync.dma_start(w_sq[:], ws_t.ap()[:])
        asq = consts.tile([P, KCH], f32)
        nc.sync.dma_start(asq[:], aq_t.ap()[:])

        # ---- stage bf16 copies of x / x_local in DRAM -----------------------
        xbf_d = dram.tile([N, D], bf16)
        xlbf_d = dram.tile([NLOC, D], bf16)
        for src, dst, rows in ((x, xbf_d, N), (xl, xlbf_d, NLOC)):
            for nb in range(rows // P):
                xt = loads.tile([P, D], f32, tag="xload")
                nc.sync.dma_start(xt[:], src[nb * P : (nb + 1) * P, :])
                xb = loads.tile([P, D], bf16, tag="xbf")
                nc.vector.tensor_copy(xb[:], xt[:])
                nc.sync.dma_start(dst[nb * P : (nb + 1) * P, :], xb[:])

        def load_xT(dst_tile, src_d, row0):
            """bf16 x^T panel: dst[:, c*512+j] = x[row0+j, c*128+partition]."""
            for c in range(CHD):
                nc.sync.dma_start(
                    dst_tile[:, c * PANEL : (c + 1) * PANEL],
                    src_d[row0 : row0 + PANEL, c * P : (c + 1) * P],
                    transpose=True,
                )

        def build_rnorm(xT_tile):
            """[4, 512] f32 tile of 1/sqrt(max(s,eps)) for this panel."""
            pn = ps_norm.tile([H, PANEL], f32)
            for c in range(CHD):
                xsq = sq.tile([P, PANEL], bf16, tag="xsq")
                src = xT_tile[:, c * PANEL : (c + 1) * PANEL]
                nc.vector.tensor_mul(xsq[:], src, src)
                nc.tensor.matmul(
                    pn[:],
                    w_sq[:, c * H : (c + 1) * H],
                    xsq[:],
                    start=(c == 0),
                    stop=(c == CHD - 1),
                )
            clamped = small.tile([H, PANEL], f32, tag="clamped")
            nc.vector.tensor_scalar_max(clamped[:], pn[:], EPS)
            root = small.tile([H, PANEL], f32, tag="root")
            nc.scalar.sqrt(root[:], clamped[:])
            rnorm = small.tile([H, PANEL], f32, tag="rnorm")
            nc.vector.reciprocal(rnorm[:], root[:])
            # Bounce through DRAM: a step-0 partition dim is only legal on
            # DRAM APs, and the DMA engines do the 128-way replication.
            rnd = dram.tile([H, PANEL], f32, tag="rnd", bufs=2)
            nc.sync.dma_start(rnd[:], rnorm[:])
            return rnd

        def bcast_rnorm(rnd, h):
            """[128, 512] f32: rnorm[h, :] replicated to all partitions."""
            bc = bcp.tile([P, PANEL], f32, tag="bc")
            nc.sync.dma_start(bc[:], rnd[h : h + 1, :].partition_broadcast(P))
            return bc

        # ---- resident lhsT: (x * a^2 * rnorm / H)^T for local rows ---------
        lhsT = consts.tile([P, KCH * NLOC], bf16)
        for lp in range(2):
            xlT = xlocp.tile([P, CHD * PANEL], bf16, tag=f"xlocT{lp}")
            load_xT(xlT, xlbf_d, lp * PANEL)
            rn = build_rnorm(xlT)
            for h in range(H):
                bc = bcast_rnorm(rn, h)
                for c in range(CHD):
                    kc = h * CHD + c
                    scaled = sq.tile([P, PANEL], f32, tag="scaled")
                    nc.vector.tensor_scalar_mul(
                        scaled[:], bc[:], asq[:, kc : kc + 1]
                    )
                    nc.vector.tensor_mul(
                        lhsT[:, kc * NLOC + lp * PANEL : kc * NLOC + (lp + 1) * PANEL],
                        xlT[:, c * PANEL : (c + 1) * PANEL],
                        scaled[:],
                    )

        # ---- main loop over 16 column panels -------------------------------
        for p in range(NPANELS):
            xT = xtp.tile([P, CHD * PANEL], bf16, tag="xT")
            load_xT(xT, xbf_d, p * PANEL)
            rn = build_rnorm(xT)

            rhs = rhsp.tile([P, KCH * PANEL], bf16)
            for h in range(H):
                bc = bcast_rnorm(rn, h)
                for c in range(CHD):
                    kc = h * CHD + c
                    nc.vector.tensor_mul(
                        rhs[:, kc * PANEL : (kc + 1) * PANEL],
                        xT[:, c * PANEL : (c + 1) * PANEL],
                        bc[:],
                    )

            for r in range(RBLK):
                acc = ps_out.tile([P, PANEL], f32)
                for kc in range(KCH):
                    nc.tensor.matmul(
                        acc[:],
                        lhsT[:, kc * NLOC + r * P : kc * NLOC + (r + 1) * P],
                        rhs[:, kc * PANEL : (kc + 1) * PANEL],
                        start=(kc == 0),
                        stop=(kc == KCH - 1),
                    )
                ot = outp.tile([P, PANEL], f32)
                nc.vector.tensor_copy(ot[:], acc[:])
                nc.sync.dma_start(
                    out[r * P : (r + 1) * P, p * PANEL : (p + 1) * PANEL], ot[:]
                )

    nc.compile()
    return nc


def _get_compiled():
    if "nc" not in _COMPILED:
        _COMPILED["nc"] = _build_bass()
    return _COMPILED["nc"]


def host_side_inputs(x, attn):
    """Per-core input maps (w_sq / asq are tiny host-precomputed functions
    of attn_vectors; see _build_bass)."""
    import ml_dtypes

    w_sq = np.zeros((P, CHD * H), dtype=np.float32)
    asq = np.zeros((P, KCH), dtype=np.float32)
    for c in range(CHD):
        w_sq[:, c * H : (c + 1) * H] = (attn[:, c * P : (c + 1) * P] ** 2).T
    for kc in range(KCH):
        h, c = divmod(kc, CHD)
        asq[:, kc] = 0.25 * attn[h, c * P : (c + 1) * P] ** 2
    w_sq = w_sq.astype(ml_dtypes.bfloat16)
    return [
        {
            "x": x,
            "x_local": np.ascontiguousarray(x[c * NLOC : (c + 1) * NLOC]),
            "w_sq": w_sq,
            "asq": asq,
        }
        for c in range(NCORES)
    ]


def kernel(**inputs) -> np.ndarray:
    from concourse import bass_utils

    x = np.ascontiguousarray(np.asarray(inputs["x"], dtype=np.float32))
    attn = np.ascontiguousarray(
        np.asarray(inputs["attn_vectors"], dtype=np.float32)
    )
    nc = _get_compiled()
    res = bass_utils.run_bass_kernel_spmd(
        nc, host_side_inputs(x, attn), core_ids=list(range(NCORES))
    )
    out = np.concatenate([r["out"] for r in res.results], axis=0)
    # The exact result is symmetric; the bf16 rounding errors of the two
    # triangles are independent, so symmetrizing averages them down.
    return ((out + out.T) * 0.5).astype(np.float32)


# revision 15
# speedup vs baseline: 3.3445x; 1.1746x over previous
"""Self-contained Trainium2 Bass kernel for the "Attentive" GNN message-passing
problem:

    x: [8192, 256] f32, attn_vectors: [4, 256] f32
    e_h = l2_normalize(attn_vectors[h] * x, axis=-1)        # [H, N, D]
    Y   = concat_h(e_h)                                     # [N, H*D]
    out = (Y @ Y.T) / H                                     # [N, N]

Strategy (8 NeuronCores, SPMD, no collectives):
  - Output rows are sharded 8 x 1024; every core receives the FULL x plus its
    own x_local row-shard as separate inputs, so the program is core-agnostic.
  - Key algebra: out[i,j] = sum_k (x*a^2*r/H)[i,k] * (x*r)[j,k] with
    r_h[n] = 1/sqrt(max(sum_d (a_h[d]*x[n,d])^2, eps)); a^2 and the 1/H are
    folded into the (small, resident) lhsT side only, so the streamed rhs
    panels need just one elementwise multiply each.
  - Everything runs in "features on partitions" layout (x^T), obtained by
    staging a bf16 copy of x in DRAM (per-panel tiles, so dependencies stay
    fine-grained) and reading it back through the DMA xbar transpose.
  - A prepass computes all row norms (cross-partition sums as tiny PE matmuls
    against host-precomputed a^2 weights) and parks rnorm rows in DRAM; the
    main loop re-broadcasts them across partitions with plain DMAs
    (step-0 partition APs are legal on DRAM sources).
  - Matmul inputs are bf16 (PE runs f32 at quarter rate); PSUM accumulates
    f32; the f32 result is copied out by DVE and DMA'd to DRAM.
"""

from contextlib import ExitStack

import numpy as np

N, D, H = 8192, 256, 4
NCORES = 8
NLOC = N // NCORES  # 1024 output rows per core
P = 128
PANEL = 512
NPANELS = N // PANEL  # 16
RBLK = NLOC // P  # 8 row blocks of the local output
KCH = (H * D) // P  # 8 contraction chunks of 128
CHD = D // P  # 2 chunks per head
EPS = 1e-12

_COMPILED = {}


def _build_bass():
    import concourse.bass as bass  # noqa: F401
    import concourse.tile as tile
    from concourse import bacc, mybir

    f32 = mybir.dt.float32
    bf16 = mybir.dt.bfloat16

    nc = bacc.Bacc(
        "TRN2",
        target_bir_lowering=False,
        debug=False,
        enable_asserts=False,
        num_devices=NCORES,
    )
    x_t = nc.dram_tensor("x", [N, D], f32, kind="ExternalInput")
    xl_t = nc.dram_tensor("x_local", [NLOC, D], f32, kind="ExternalInput")
    # Host-precomputed functions of attn_vectors (tiny):
    #   w_sq[d, c*4+h]  = attn[h, c*128+d]^2          (bf16, norm matmul lhsT)
    #   asq[d, kc]      = 0.25*attn[h, c*128+d]^2     (f32, kc = h*2+c)
    ws_t = nc.dram_tensor("w_sq", [P, CHD * H], bf16, kind="ExternalInput")
    aq_t = nc.dram_tensor("asq", [P, KCH], f32, kind="ExternalInput")
    out_t = nc.dram_tensor("out", [NLOC, N], f32, kind="ExternalOutput")

    x, xl, out = x_t.ap(), xl_t.ap(), out_t.ap()

    with tile.TileContext(nc) as tc, ExitStack() as ctx:
        consts = ctx.enter_context(tc.tile_pool(name="consts", bufs=1))
        loads = ctx.enter_context(tc.tile_pool(name="loads", bufs=6))
        xtp = ctx.enter_context(tc.tile_pool(name="xtp", bufs=1))
        sq = ctx.enter_context(tc.tile_pool(name="sq", bufs=4))
        small = ctx.enter_context(tc.tile_pool(name="small", bufs=3))
        bcp = ctx.enter_context(tc.tile_pool(name="bcp", bufs=6))
        rhsp = ctx.enter_context(tc.tile_pool(name="rhsp", bufs=3))
        outp = ctx.enter_context(tc.tile_pool(name="outp", bufs=6))
        dram = ctx.enter_context(tc.tile_pool(name="dram", bufs=1, space="DRAM"))
        ps_norm = ctx.enter_context(
            tc.tile_pool(name="ps_norm", bufs=2, space="PSUM")
        )
        ps_out = ctx.enter_context(
            tc.tile_pool(name="ps_out", bufs=6, space="PSUM")
        )

        w_sq = consts.tile([P, CHD * H], bf16)
        nc.sync.dma_start(w_sq[:], ws_t.ap()[:])
        asq = consts.tile([P, KCH], f32)
        nc.sync.dma_start(asq[:], aq_t.ap()[:])

        def stage_panel(src_ap, row0, name):
            """bf16 copy of 512 source rows parked in its own DRAM tile."""
            d = dram.tile([PANEL, D], bf16, name=name)
            for i in range(PANEL // P):
                xt = loads.tile([P, D], f32, tag="xload")
                nc.sync.dma_start(
                    xt[:], src_ap[row0 + i * P : row0 + (i + 1) * P, :]
                )
                xb = loads.tile([P, D], bf16, tag="xbf")
                nc.gpsimd.tensor_copy(xb[:], xt[:])
                nc.sync.dma_start(d[i * P : (i + 1) * P, :], xb[:])
            return d

        def prepass(stage_d, xT_tile, name):
            """DMA-transpose a staged panel into SBUF and park this panel's
            rnorm rows ([4, 512] f32 of 1/sqrt(max(s,eps))) in DRAM."""
            for c in range(CHD):
                nc.sync.dma_start(
                    xT_tile[:, c * PANEL : (c + 1) * PANEL],
                    stage_d[:, c * P : (c + 1) * P],
                    transpose=True,
                )
            pn = ps_norm.tile([H, PANEL], f32, tag="pn")
            for c in range(CHD):
                xsq = sq.tile([P, PANEL], bf16, tag="xsq")
                src = xT_tile[:, c * PANEL : (c + 1) * PANEL]
                nc.vector.tensor_mul(xsq[:], src, src)
                nc.tensor.matmul(
                    pn[:],
                    w_sq[:, c * H : (c + 1) * H],
                    xsq[:],
                    start=(c == 0),
                    stop=(c == CHD - 1),
                )
            clamped = small.tile([H, PANEL], f32, tag="clamped")
            nc.vector.tensor_scalar_max(clamped[:], pn[:], EPS)
            root = small.tile([H, PANEL], f32, tag="root")
            nc.scalar.sqrt(root[:], clamped[:])
            rnorm = small.tile([H, PANEL], f32, tag="rnorm")
            nc.vector.reciprocal(rnorm[:], root[:])
            rnd = dram.tile([H, PANEL], f32, name=name)
            nc.sync.dma_start(rnd[:], rnorm[:])
            return rnd

        def bcast_rnorm(rnd, h):
            """[128, 512] f32: rnorm[h, :] replicated to all partitions."""
            bc = bcp.tile([P, PANEL], f32, tag="bc")
            nc.sync.dma_start(bc[:], rnd[h : h + 1, :].partition_broadcast(P))
            return bc

        # ---- local rows: stage, prepass, resident lhsT ---------------------
        lhsT = consts.tile([P, KCH * NLOC], bf16)
        xlocT = []
        lrnd = []
        for lp in range(2):
            d = stage_panel(xl, lp * PANEL, f"xlbf{lp}")
            t = consts.tile([P, CHD * PANEL], bf16, name=f"xlocT{lp}")
            xlocT.append(t)
            lrnd.append(prepass(d, t, f"lrnd{lp}"))
        for lp in range(2):
            for h in range(H):
                bc = bcast_rnorm(lrnd[lp], h)
                for c in range(CHD):
                    kc = h * CHD + c
                    scaled = sq.tile([P, PANEL], f32, tag="scaled")
                    nc.vector.tensor_scalar_mul(
                        scaled[:], bc[:], asq[:, kc : kc + 1]
                    )
                    nc.vector.tensor_mul(
                        lhsT[
                            :,
                            kc * NLOC + lp * PANEL : kc * NLOC + (lp + 1) * PANEL,
                        ],
                        xlocT[lp][:, c * PANEL : (c + 1) * PANEL],
                        scaled[:],
                    )

        # ---- full x: stage + prepass (xT panels stay resident) -------------
        xTs = []
        rnds = []
        for p in range(NPANELS):
            d = stage_panel(x, p * PANEL, f"xbf{p}")
            t = xtp.tile([P, CHD * PANEL], bf16, name=f"xT{p}")
            xTs.append(t)
            rnds.append(prepass(d, t, f"rnd{p}"))

        # ---- main loop over 16 column panels -------------------------------
        for p in range(NPANELS):
            rhs = rhsp.tile([P, KCH * PANEL], bf16, tag="rhs")
            for h in range(H):
                bc = bcast_rnorm(rnds[p], h)
                for c in range(CHD):
                    kc = h * CHD + c
                    nc.vector.tensor_mul(
                        rhs[:, kc * PANEL : (kc + 1) * PANEL],
                        xTs[p][:, c * PANEL : (c + 1) * PANEL],
                        bc[:],
                    )

            for r in range(RBLK):
                acc = ps_out.tile([P, PANEL], f32, tag="acc")
                for kc in range(KCH):
                    nc.tensor.matmul(
                        acc[:],
                        lhsT[:, kc * NLOC + r * P : kc * NLOC + (r + 1) * P],
                        rhs[:, kc * PANEL : (kc + 1) * PANEL],
                        start=(kc == 0),
                        stop=(kc == KCH - 1),
                    )
                ot = outp.tile([P, PANEL], f32, tag="ot")
                nc.vector.tensor_copy(ot[:], acc[:])
                nc.sync.dma_start(
                    out[r * P : (r + 1) * P, p * PANEL : (p + 1) * PANEL], ot[:]
                )

    nc.compile()
    return nc


def _get_compiled():
    if "nc" not in _COMPILED:
        _COMPILED["nc"] = _build_bass()
    return _COMPILED["nc"]


def host_side_inputs(x, attn):
    """Per-core input maps (w_sq / asq are tiny host-precomputed functions
    of attn_vectors; see _build_bass)."""
    import ml_dtypes

    w_sq = np.zeros((P, CHD * H), dtype=np.float32)
    asq = np.zeros((P, KCH), dtype=np.float32)
    for c in range(CHD):
        w_sq[:, c * H : (c + 1) * H] = (attn[:, c * P : (c + 1) * P] ** 2).T
    for kc in range(KCH):
        h, c = divmod(kc, CHD)
        asq[:, kc] = 0.25 * attn[h, c * P : (c + 1) * P] ** 2
    w_sq = w_sq.astype(ml_dtypes.bfloat16)
    return [
        {
            "x": x,
            "x_local": np.ascontiguousarray(x[c * NLOC : (c + 1) * NLOC]),
            "w_sq": w_sq,
            "asq": asq,
        }
        for c in range(NCORES)
    ]


def kernel(**inputs) -> np.ndarray:
    from concourse import bass_utils

    x = np.ascontiguousarray(np.asarray(inputs["x"], dtype=np.float32))
    attn = np.ascontiguousarray(
        np.asarray(inputs["attn_vectors"], dtype=np.float32)
    )
    nc = _get_compiled()
    res = bass_utils.run_bass_kernel_spmd(
        nc, host_side_inputs(x, attn), core_ids=list(range(NCORES))
    )
    out = np.concatenate([r["out"] for r in res.results], axis=0)
    # The exact result is symmetric; the bf16 rounding errors of the two
    # triangles are independent, so symmetrizing averages them down.
    return ((out + out.T) * 0.5).astype(np.float32)


# revision 21
# speedup vs baseline: 3.4057x; 1.0183x over previous
"""Self-contained Trainium2 Bass kernel for the "Attentive" GNN message-passing
problem:

    x: [8192, 256] f32, attn_vectors: [4, 256] f32
    e_h = l2_normalize(attn_vectors[h] * x, axis=-1)        # [H, N, D]
    Y   = concat_h(e_h)                                     # [N, H*D]
    out = (Y @ Y.T) / H                                     # [N, N]

Strategy (8 NeuronCores, SPMD, no collectives):
  - Output rows are sharded 8 x 1024; every core receives the FULL x plus its
    own x_local row-shard as separate inputs, so the program is core-agnostic.
  - Key algebra: out[i,j] = sum_k (x*a^2*r/H)[i,k] * (x*r)[j,k] with
    r_h[n] = 1/sqrt(max(sum_d (a_h[d]*x[n,d])^2, eps)); a^2 and the 1/H are
    folded into the (small, resident) lhsT side only, so the streamed rhs
    panels need just one elementwise multiply each.
  - Everything runs in "features on partitions" layout (x^T), obtained by
    staging a bf16 copy of x in DRAM (per-panel tiles, fine-grained deps)
    and reading it back through the DMA xbar transpose.
  - Row norms are computed as transposed PE matmuls xsq^T @ a^2 so the
    max/sqrt/reciprocal chain runs in [128, 16] layout (all DVE lanes
    active); rnorm rows bounce through DRAM and come back as one batched
    broadcast DMA per panel (step-0 partition APs are legal on DRAM).
  - Matmul inputs are bf16 (PE runs f32 at quarter rate); PSUM accumulates
    f32; each panel's 8 PSUM tiles are copied into one SBUF tile and leave
    in a single 2 MB DMA.
  - DMAs are batched aggressively: the SP sequencer pays ~600 ns per
    dma_start, so the panel pipeline uses ~7 DMAs per 512-column panel.
"""

from contextlib import ExitStack

import numpy as np

N, D, H = 8192, 256, 4
NCORES = 8
NLOC = N // NCORES  # 1024 output rows per core
P = 128
PANEL = 512
NPANELS = N // PANEL  # 16
RBLK = NLOC // P  # 8 row blocks of the local output
KCH = (H * D) // P  # 8 contraction chunks of 128
CHD = D // P  # 2 chunks per head
SUB = PANEL // P  # 4 column sub-blocks per panel
EPS = 1e-12

_COMPILED = {}


def _build_bass():
    import concourse.bass as bass
    import concourse.tile as tile
    from concourse import bacc, mybir

    f32 = mybir.dt.float32
    bf16 = mybir.dt.bfloat16

    nc = bacc.Bacc(
        "TRN2",
        target_bir_lowering=False,
        debug=False,
        enable_asserts=False,
        num_devices=NCORES,
    )
    x_t = nc.dram_tensor("x", [N, D], f32, kind="ExternalInput")
    xl_t = nc.dram_tensor("x_local", [NLOC, D], f32, kind="ExternalInput")
    # Host-precomputed functions of attn_vectors (tiny):
    #   w_sq[d, c*4+h]  = attn[h, c*128+d]^2          (bf16, norm matmul rhs)
    #   asq[d, kc]      = 0.25*attn[h, c*128+d]^2     (f32, kc = h*2+c)
    ws_t = nc.dram_tensor("w_sq", [P, CHD * H], bf16, kind="ExternalInput")
    aq_t = nc.dram_tensor("asq", [P, KCH], f32, kind="ExternalInput")
    out_t = nc.dram_tensor("out", [NLOC, N], f32, kind="ExternalOutput")

    x, xl, out = x_t.ap(), xl_t.ap(), out_t.ap()

    with tile.TileContext(nc) as tc, ExitStack() as ctx:
        consts = ctx.enter_context(tc.tile_pool(name="consts", bufs=1))
        loads = ctx.enter_context(tc.tile_pool(name="loads", bufs=3))
        xtp = ctx.enter_context(tc.tile_pool(name="xtp", bufs=1))
        sq = ctx.enter_context(tc.tile_pool(name="sq", bufs=4))
        small = ctx.enter_context(tc.tile_pool(name="small", bufs=3))
        bcp = ctx.enter_context(tc.tile_pool(name="bcp", bufs=3))
        rhsp = ctx.enter_context(tc.tile_pool(name="rhsp", bufs=3))
        outp = ctx.enter_context(tc.tile_pool(name="outp", bufs=2))
        dram = ctx.enter_context(tc.tile_pool(name="dram", bufs=1, space="DRAM"))
        ps_norm = ctx.enter_context(
            tc.tile_pool(name="ps_norm", bufs=2, space="PSUM")
        )
        ps_out = ctx.enter_context(
            tc.tile_pool(name="ps_out", bufs=4, space="PSUM")
        )

        from concourse.masks import make_identity

        w_sq = consts.tile([P, CHD * H], bf16)
        nc.sync.dma_start(w_sq[:], ws_t.ap()[:])
        asq = consts.tile([P, KCH], f32)
        nc.sync.dma_start(asq[:], aq_t.ap()[:])
        ident = consts.tile([P, P], f32)
        make_identity(nc, ident[:])

        def stage_panel(src_ap, row0, name):
            """bf16 copy of 512 source rows parked in its own DRAM tile.
            One batched load / cast / store: SBUF column i*D+d holds source
            row row0+i*128+q at partition q."""
            d = dram.tile([PANEL, D], bf16, name=name)
            xt = loads.tile([P, SUB * D], f32, tag="xload")
            nc.sync.dma_start(sb_rearr(xt), x_rearr(src_ap, row0))
            xb = loads.tile([P, SUB * D], bf16, tag="xbf")
            nc.gpsimd.tensor_copy(xb[:], xt[:])
            nc.sync.dma_start(x_rearr(d[:, :], 0), sb_rearr(xb))
            return d

        def sb_rearr(tile_ap):
            return tile_ap[:].rearrange("q (i d) -> q i d", i=SUB)

        def x_rearr(ap, row0):
            return ap[row0 : row0 + PANEL, :].rearrange(
                "(i q) d -> q i d", q=P
            )

        def prepass(stage_d, xT_tile, name):
            """DMA-transpose a staged panel into SBUF and park this panel's
            rnorm in DRAM as [128, 16]: rn[q, i*4+h] for column i*128+q."""
            for c in range(CHD):
                nc.sync.dma_start(
                    xT_tile[:, c * PANEL : (c + 1) * PANEL],
                    stage_d[:, c * P : (c + 1) * P],
                    transpose=True,
                )
            pn = ps_norm.tile([P, SUB * H], f32, tag="pn")
            xsqs = []
            for c in range(CHD):
                xsq = sq.tile([P, PANEL], bf16, tag=f"xsq{c}")
                src = xT_tile[:, c * PANEL : (c + 1) * PANEL]
                nc.vector.tensor_mul(xsq[:], src, src)
                xsqs.append(xsq)
            for i in range(SUB):
                for c in range(CHD):
                    nc.tensor.matmul(
                        pn[:, i * H : (i + 1) * H],
                        xsqs[c][:, i * P : (i + 1) * P],
                        w_sq[:, c * H : (c + 1) * H],
                        start=(c == 0),
                        stop=(c == CHD - 1),
                    )
            # eps-clamp; the input AP also permutes [q,(i h)] -> [q,(h i)]
            # so that after the PE transpose the store is contiguous.
            clamped = small.tile([P, SUB * H], f32, tag="clamped")
            nc.vector.tensor_scalar_max(
                clamped[:],
                pn[:].rearrange("q (i h) -> q h i", h=H),
                EPS,
            )
            root = small.tile([P, SUB * H], f32, tag="root")
            nc.scalar.sqrt(root[:], clamped[:])
            rnorm = small.tile([P, SUB * H], f32, tag="rnorm")
            nc.vector.reciprocal(rnorm[:], root[:])
            # [128, 16] -> [16, 128]; row j = h*4+i, so the flat DRAM tile
            # is rnorm_h[i*128+q] at offset h*512 + i*128 + q (h-major).
            pt = ps_norm.tile([SUB * H, P], f32, tag="pt")
            nc.tensor.transpose(pt[:], rnorm[:], ident[:])
            rno = small.tile([SUB * H, P], f32, tag="rno")
            nc.vector.tensor_copy(rno[:], pt[:])
            rnd = dram.tile([SUB * H, P], f32, name=name)
            nc.sync.dma_start(rnd[:], rno[:])
            return rnd

        def bcast_rnorm(rnd):
            """[128, 4*512] f32: bc[:, h*512 + n] = rnorm_h[n], one DMA."""
            bc = bcp.tile([P, H * PANEL], f32, tag="bc")
            src = bass.AP(
                rnd.tensor,
                rnd.offset,
                [[0, P], [PANEL, H], [1, PANEL]],
            )
            nc.sync.dma_start(
                bc[:].rearrange("p (h n) -> p h n", h=H), src
            )
            return bc

        # ---- local rows: stage, prepass, resident lhsT ---------------------
        lhsT = consts.tile([P, KCH * NLOC], bf16)
        for lp in range(2):
            d = stage_panel(xl, lp * PANEL, f"xlbf{lp}")
            t = consts.tile([P, CHD * PANEL], bf16, name=f"xlocT{lp}")
            rnd = prepass(d, t, f"lrnd{lp}")
            bc = bcast_rnorm(rnd)
            for h in range(H):
                for c in range(CHD):
                    kc = h * CHD + c
                    scaled = sq.tile([P, PANEL], f32, tag="scaled")
                    nc.vector.tensor_scalar_mul(
                        scaled[:],
                        bc[:, h * PANEL : (h + 1) * PANEL],
                        asq[:, kc : kc + 1],
                    )
                    nc.vector.tensor_mul(
                        lhsT[
                            :,
                            kc * NLOC + lp * PANEL : kc * NLOC + (lp + 1) * PANEL,
                        ],
                        t[:, c * PANEL : (c + 1) * PANEL],
                        scaled[:],
                    )

        # ---- full x: stage + prepass (xT panels stay resident) -------------
        xTs = []
        rnds = []
        for p in range(NPANELS):
            d = stage_panel(x, p * PANEL, f"xbf{p}")
            t = xtp.tile([P, CHD * PANEL], bf16, name=f"xT{p}")
            xTs.append(t)
            rnds.append(prepass(d, t, f"rnd{p}"))

        # ---- main loop over 16 column panels -------------------------------
        for p in range(NPANELS):
            bc = bcast_rnorm(rnds[p])
            rhs = rhsp.tile([P, KCH * PANEL], bf16, tag="rhs")
            for h in range(H):
                for c in range(CHD):
                    kc = h * CHD + c
                    nc.vector.tensor_mul(
                        rhs[:, kc * PANEL : (kc + 1) * PANEL],
                        xTs[p][:, c * PANEL : (c + 1) * PANEL],
                        bc[:, h * PANEL : (h + 1) * PANEL],
                    )

            ot = outp.tile([P, RBLK * PANEL], f32, tag="ot")
            for r in range(RBLK):
                acc = ps_out.tile([P, PANEL], f32, tag="acc")
                for kc in range(KCH):
                    nc.tensor.matmul(
                        acc[:],
                        lhsT[:, kc * NLOC + r * P : kc * NLOC + (r + 1) * P],
                        rhs[:, kc * PANEL : (kc + 1) * PANEL],
                        start=(kc == 0),
                        stop=(kc == KCH - 1),
                    )
                nc.vector.tensor_copy(
                    ot[:, r * PANEL : (r + 1) * PANEL], acc[:]
                )
            nc.sync.dma_start(
                out[:, p * PANEL : (p + 1) * PANEL].rearrange(
                    "(r q) c -> q r c", q=P
                ),
                ot[:].rearrange("q (r c) -> q r c", r=RBLK),
            )

    nc.compile()
    return nc


def _get_compiled():
    if "nc" not in _COMPILED:
        _COMPILED["nc"] = _build_bass()
    return _COMPILED["nc"]


def host_side_inputs(x, attn):
    """Per-core input maps (w_sq / asq are tiny host-precomputed functions
    of attn_vectors; see _build_bass)."""
    import ml_dtypes

    w_sq = np.zeros((P, CHD * H), dtype=np.float32)
    asq = np.zeros((P, KCH), dtype=np.float32)
    for c in range(CHD):
        w_sq[:, c * H : (c + 1) * H] = (attn[:, c * P : (c + 1) * P] ** 2).T
    for kc in range(KCH):
        h, c = divmod(kc, CHD)
        asq[:, kc] = 0.25 * attn[h, c * P : (c + 1) * P] ** 2
    w_sq = w_sq.astype(ml_dtypes.bfloat16)
    return [
        {
            "x": x,
            "x_local": np.ascontiguousarray(x[c * NLOC : (c + 1) * NLOC]),
            "w_sq": w_sq,
            "asq": asq,
        }
        for c in range(NCORES)
    ]


def kernel(**inputs) -> np.ndarray:
    from concourse import bass_utils

    x = np.ascontiguousarray(np.asarray(inputs["x"], dtype=np.float32))
    attn = np.ascontiguousarray(
        np.asarray(inputs["attn_vectors"], dtype=np.float32)
    )
    nc = _get_compiled()
    res = bass_utils.run_bass_kernel_spmd(
        nc, host_side_inputs(x, attn), core_ids=list(range(NCORES))
    )
    out = np.concatenate([r["out"] for r in res.results], axis=0)
    # The exact result is symmetric; the bf16 rounding errors of the two
    # triangles are independent, so symmetrizing averages them down.
    return ((out + out.T) * 0.5).astype(np.float32)


# revision 25
# speedup vs baseline: 5.3453x; 1.5695x over previous
"""Self-contained Trainium2 Bass kernel for the "Attentive" GNN message-passing
problem:

    x: [8192, 256] f32, attn_vectors: [4, 256] f32
    e_h = l2_normalize(attn_vectors[h] * x, axis=-1)        # [H, N, D]
    Y   = concat_h(e_h)                                     # [N, H*D]
    out = (Y @ Y.T) / H                                     # [N, N]

Strategy (8 NeuronCores, SPMD, no collectives):
  - Output rows are sharded 8 x 1024; every core receives the FULL x plus its
    own x_local row-shard as separate inputs, so the program is core-agnostic.
  - Key algebra: out[i,j] = sum_k (x*a^2*r/H)[i,k] * (x*r)[j,k] with
    r_h[n] = 1/sqrt(max(sum_d (a_h[d]*x[n,d])^2, eps)); a^2 and the 1/H are
    folded into the (small, resident) lhsT side only, so the streamed rhs
    panels need just one elementwise multiply each.
  - Everything runs in "features on partitions" layout (x^T), obtained by
    staging a bf16 copy of x in DRAM (per-panel tiles, fine-grained deps)
    and reading it back through the DMA xbar transpose.
  - Row norms are computed as transposed PE matmuls xsq^T @ a^2 so the
    max/sqrt/reciprocal chain runs in [128, 16] layout (all DVE lanes
    active); rnorm rows bounce through DRAM and come back as one batched
    broadcast DMA per panel (step-0 partition APs are legal on DRAM).
  - Matmul inputs are bf16 (PE runs f32 at quarter rate); PSUM accumulates
    f32; each panel's 8 PSUM tiles are copied into one SBUF tile and leave
    in a single 2 MB DMA.
  - DMAs are batched aggressively: the SP sequencer pays ~600 ns per
    dma_start, so the panel pipeline uses ~7 DMAs per 512-column panel.
"""

from contextlib import ExitStack

import numpy as np

N, D, H = 8192, 256, 4
NCORES = 8
NLOC = N // NCORES  # 1024 output rows per core
P = 128
PANEL = 512
NPANELS = N // PANEL  # 16
RBLK = NLOC // P  # 8 row blocks of the local output
KCH = (H * D) // P  # 8 contraction chunks of 128
CHD = D // P  # 2 chunks per head
SUB = PANEL // P  # 4 column sub-blocks per panel
EPS = 1e-12

_COMPILED = {}


def _build_bass():
    import concourse.bass as bass
    import concourse.tile as tile
    from concourse import bacc, mybir

    f32 = mybir.dt.float32
    bf16 = mybir.dt.bfloat16

    nc = bacc.Bacc(
        "TRN2",
        target_bir_lowering=False,
        debug=False,
        enable_asserts=False,
        num_devices=NCORES,
    )
    x_t = nc.dram_tensor("x", [N, D], f32, kind="ExternalInput")
    xl_t = nc.dram_tensor("x_local", [NLOC, D], f32, kind="ExternalInput")
    # Host-precomputed functions of attn_vectors (tiny):
    #   w_sq[d, c*4+h]  = attn[h, c*128+d]^2          (bf16, norm matmul rhs)
    #   asq[d, kc]      = 0.25*attn[h, c*128+d]^2     (f32, kc = h*2+c)
    ws_t = nc.dram_tensor("w_sq", [P, CHD * H], bf16, kind="ExternalInput")
    aq_t = nc.dram_tensor("asq", [P, KCH], f32, kind="ExternalInput")
    out_t = nc.dram_tensor("out", [NLOC, N], f32, kind="ExternalOutput")

    x, xl, out = x_t.ap(), xl_t.ap(), out_t.ap()

    with tile.TileContext(nc) as tc, ExitStack() as ctx:
        consts = ctx.enter_context(tc.tile_pool(name="consts", bufs=1))
        loads = ctx.enter_context(tc.tile_pool(name="loads", bufs=3))
        xtp = ctx.enter_context(tc.tile_pool(name="xtp", bufs=1))
        sq = ctx.enter_context(tc.tile_pool(name="sq", bufs=4))
        small = ctx.enter_context(tc.tile_pool(name="small", bufs=3))
        bcp = ctx.enter_context(tc.tile_pool(name="bcp", bufs=3))
        rhsp = ctx.enter_context(tc.tile_pool(name="rhsp", bufs=3))
        outp = ctx.enter_context(tc.tile_pool(name="outp", bufs=2))
        dram = ctx.enter_context(tc.tile_pool(name="dram", bufs=1, space="DRAM"))
        ps_norm = ctx.enter_context(
            tc.tile_pool(name="ps_norm", bufs=2, space="PSUM")
        )
        ps_out = ctx.enter_context(
            tc.tile_pool(name="ps_out", bufs=4, space="PSUM")
        )

        from concourse.masks import make_identity

        w_sq = consts.tile([P, CHD * H], bf16)
        nc.sync.dma_start(w_sq[:], ws_t.ap()[:])
        asq = consts.tile([P, KCH], f32)
        nc.sync.dma_start(asq[:], aq_t.ap()[:])
        ident = consts.tile([P, P], f32)
        make_identity(nc, ident[:])

        def sb_rearr(tile_ap):
            return tile_ap[:].rearrange("q (i d) -> q i d", i=SUB)

        def x_rearr(ap, row0):
            return ap[row0 : row0 + PANEL, :].rearrange(
                "(i q) d -> q i d", q=P
            )

        def prepass(src_ap, row0, xT_tile, name):
            """Load 512 source rows (one batched DMA), transpose them on the
            PE into bf16 x^T, and park this panel's rnorm in DRAM.
            The PSUM->SBUF copy after each transpose doubles as the f32->bf16
            cast."""
            xt = loads.tile([P, SUB * D], f32, tag="xload")
            nc.sync.dma_start(sb_rearr(xt), x_rearr(src_ap, row0))
            for i in range(SUB):
                for c in range(CHD):
                    tp = ps_norm.tile([P, P], f32, tag="tp")
                    nc.tensor.transpose(
                        tp[:], xt[:, i * D + c * P : i * D + (c + 1) * P], ident[:]
                    )
                    nc.vector.tensor_copy(
                        xT_tile[:, c * PANEL + i * P : c * PANEL + (i + 1) * P],
                        tp[:],
                    )
            pn = ps_norm.tile([P, SUB * H], f32, tag="pn")
            xsqs = []
            for c in range(CHD):
                xsq = sq.tile([P, PANEL], bf16, tag=f"xsq{c}")
                src = xT_tile[:, c * PANEL : (c + 1) * PANEL]
                nc.vector.tensor_mul(xsq[:], src, src)
                xsqs.append(xsq)
            for i in range(SUB):
                for c in range(CHD):
                    nc.tensor.matmul(
                        pn[:, i * H : (i + 1) * H],
                        xsqs[c][:, i * P : (i + 1) * P],
                        w_sq[:, c * H : (c + 1) * H],
                        start=(c == 0),
                        stop=(c == CHD - 1),
                    )
            # eps-clamp; the input AP also permutes [q,(i h)] -> [q,(h i)]
            # so that after the PE transpose the store is contiguous.
            clamped = small.tile([P, SUB * H], f32, tag="clamped")
            nc.vector.tensor_scalar_max(
                clamped[:],
                pn[:].rearrange("q (i h) -> q h i", h=H),
                EPS,
            )
            root = small.tile([P, SUB * H], f32, tag="root")
            nc.scalar.sqrt(root[:], clamped[:])
            rnorm = small.tile([P, SUB * H], f32, tag="rnorm")
            nc.vector.reciprocal(rnorm[:], root[:])
            # [128, 16] -> [16, 128]; row j = h*4+i, so the flat DRAM tile
            # is rnorm_h[i*128+q] at offset h*512 + i*128 + q (h-major).
            pt = ps_norm.tile([SUB * H, P], f32, tag="tp")
            nc.tensor.transpose(pt[:], rnorm[:], ident[:])
            rno = small.tile([SUB * H, P], f32, tag="rno")
            nc.vector.tensor_copy(rno[:], pt[:])
            rnd = dram.tile([SUB * H, P], f32, name=name)
            nc.sync.dma_start(rnd[:], rno[:])
            return rnd

        def bcast_rnorm(rnd):
            """[128, 4*512] f32: bc[:, h*512 + n] = rnorm_h[n], one DMA."""
            bc = bcp.tile([P, H * PANEL], f32, tag="bc")
            src = bass.AP(
                rnd.tensor,
                rnd.offset,
                [[0, P], [PANEL, H], [1, PANEL]],
            )
            nc.sync.dma_start(
                bc[:].rearrange("p (h n) -> p h n", h=H), src
            )
            return bc

        # ---- local rows: prepass + resident lhsT ---------------------------
        lhsT = consts.tile([P, KCH * NLOC], bf16)
        for lp in range(2):
            t = consts.tile([P, CHD * PANEL], bf16, name=f"xlocT{lp}")
            rnd = prepass(xl, lp * PANEL, t, f"lrnd{lp}")
            bc = bcast_rnorm(rnd)
            for h in range(H):
                for c in range(CHD):
                    kc = h * CHD + c
                    scaled = sq.tile([P, PANEL], f32, tag="scaled")
                    nc.vector.tensor_scalar_mul(
                        scaled[:],
                        bc[:, h * PANEL : (h + 1) * PANEL],
                        asq[:, kc : kc + 1],
                    )
                    nc.vector.tensor_mul(
                        lhsT[
                            :,
                            kc * NLOC + lp * PANEL : kc * NLOC + (lp + 1) * PANEL,
                        ],
                        t[:, c * PANEL : (c + 1) * PANEL],
                        scaled[:],
                    )

        # ---- full x: prepass (xT panels stay resident) ---------------------
        xTs = []
        rnds = []
        for p in range(NPANELS):
            t = xtp.tile([P, CHD * PANEL], bf16, name=f"xT{p}")
            xTs.append(t)
            rnds.append(prepass(x, p * PANEL, t, f"rnd{p}"))

        # ---- main loop over 16 column panels -------------------------------
        for p in range(NPANELS):
            bc = bcast_rnorm(rnds[p])
            rhs = rhsp.tile([P, KCH * PANEL], bf16, tag="rhs")
            for h in range(H):
                for c in range(CHD):
                    kc = h * CHD + c
                    nc.vector.tensor_mul(
                        rhs[:, kc * PANEL : (kc + 1) * PANEL],
                        xTs[p][:, c * PANEL : (c + 1) * PANEL],
                        bc[:, h * PANEL : (h + 1) * PANEL],
                    )

            ot = outp.tile([P, RBLK * PANEL], f32, tag="ot")
            for r in range(RBLK):
                acc = ps_out.tile([P, PANEL], f32, tag="acc")
                for kc in range(KCH):
                    nc.tensor.matmul(
                        acc[:],
                        lhsT[:, kc * NLOC + r * P : kc * NLOC + (r + 1) * P],
                        rhs[:, kc * PANEL : (kc + 1) * PANEL],
                        start=(kc == 0),
                        stop=(kc == KCH - 1),
                    )
                nc.vector.tensor_copy(
                    ot[:, r * PANEL : (r + 1) * PANEL], acc[:]
                )
            nc.sync.dma_start(
                out[:, p * PANEL : (p + 1) * PANEL].rearrange(
                    "(r q) c -> q r c", q=P
                ),
                ot[:].rearrange("q (r c) -> q r c", r=RBLK),
            )

    nc.compile()
    return nc


def _get_compiled():
    if "nc" not in _COMPILED:
        _COMPILED["nc"] = _build_bass()
    return _COMPILED["nc"]


def host_side_inputs(x, attn):
    """Per-core input maps (w_sq / asq are tiny host-precomputed functions
    of attn_vectors; see _build_bass)."""
    import ml_dtypes

    w_sq = np.zeros((P, CHD * H), dtype=np.float32)
    asq = np.zeros((P, KCH), dtype=np.float32)
    for c in range(CHD):
        w_sq[:, c * H : (c + 1) * H] = (attn[:, c * P : (c + 1) * P] ** 2).T
    for kc in range(KCH):
        h, c = divmod(kc, CHD)
        asq[:, kc] = 0.25 * attn[h, c * P : (c + 1) * P] ** 2
    w_sq = w_sq.astype(ml_dtypes.bfloat16)
    return [
        {
            "x": x,
            "x_local": np.ascontiguousarray(x[c * NLOC : (c + 1) * NLOC]),
            "w_sq": w_sq,
            "asq": asq,
        }
        for c in range(NCORES)
    ]


def kernel(**inputs) -> np.ndarray:
    from concourse import bass_utils

    x = np.ascontiguousarray(np.asarray(inputs["x"], dtype=np.float32))
    attn = np.ascontiguousarray(
        np.asarray(inputs["attn_vectors"], dtype=np.float32)
    )
    nc = _get_compiled()
    res = bass_utils.run_bass_kernel_spmd(
        nc, host_side_inputs(x, attn), core_ids=list(range(NCORES))
    )
    out = np.concatenate([r["out"] for r in res.results], axis=0)
    # The exact result is symmetric; the bf16 rounding errors of the two
    # triangles are independent, so symmetrizing averages them down.
    return ((out + out.T) * 0.5).astype(np.float32)
